# revision 2
# baseline (speedup 1.0000x reference)
"""LGA3 v3: banded-matmul on the Tensor engine, pass-fused row pipeline.

Idea: for one output row y, x-block X0=124c, row-offset i, disparity-group g,
the 5 x-taps form a banded [k=128, m=128] stationary matrix
    B[k=m+j, m] = w[g, i, j, y, X0+m]   (m<=123, X0+m<768; else 0)
and   out[m, d] += B.T @ slab_row[y+i][k = X0-2+.., d+g-1]
so the 15 (i,g) matmuls accumulate one [128 x, 64 d] output block in fp32
PSUM.  Bands are prebuilt on host (one fp16 image, streamed once), the three
LGA passes are fused into a 3-stage row pipeline so each band row is loaded
from DRAM exactly once and reused by all 3 passes while resident in SBUF.

Layouts (per core, H sharded 8 ways, 48 rows/core + halo):
  a     [2, 60, 772, 66] f16  input slab: rows s-6..s+53, x+2 pad, d+1 pad
  bands [2, 56, 128, 7*15*128] f16  band row ybidx ~ global s-4+ybidx,
        free dim = (c, ig=i*3+g, m); one partition's row is contiguous 26.9KB
  o     [2, 48, 768, 64] f16
Intermediate pass outputs live in SBUF rings of 7 x-tiles [128, 66]
(tile c = slab-x [124c, 124c+128)), assembled from PSUM via partition-sliced
SBUF->SBUF DMAs (main 124 cols + 2-col halo pairs into neighbor tiles).
"""

import os
import sys

for _p in ("/opt/trn_rl_repo", "/root/.axon_site/_ro/trn_rl_repo"):
    if os.path.isdir(_p) and _p not in sys.path:
        sys.path.append(_p)

import numpy as np
import concourse.bass as bass
import concourse.mybir as mybir
from concourse.tile import TileContext
from concourse import bass_utils

F16 = mybir.dt.float16
F32 = mybir.dt.float32
F8 = mybir.dt.float8e4
WSCALE = 64.0  # bands pre-scaled by this (keeps e4m3 out of denormals)


def _install_ntff_shim():
    """The agent image's antenv lacks axon_hooks, so bass_utils silently
    skips NTFF tracing. Recreate the ctypes hook (same C ABI as
    trn_agent_boot) and inject it as antenv.axon_hooks."""
    try:
        import antenv.axon_hooks  # noqa: F401
        return
    except ImportError:
        pass
    import contextlib
    import ctypes
    import types

    so_path = "/opt/axon/libaxon_pjrt.so"
    if not os.path.exists(so_path):
        return
    try:
        lib = ctypes.CDLL(so_path)
    except OSError:
        return
    if not hasattr(lib, "axon_start_nrt_profile"):
        return
    lib.axon_start_nrt_profile.argtypes = [
        ctypes.POINTER(ctypes.c_int64),
        ctypes.c_size_t,
    ]
    lib.axon_start_nrt_profile.restype = ctypes.c_int64
    lib.axon_stop_nrt_profile.argtypes = [ctypes.c_char_p]
    lib.axon_stop_nrt_profile.restype = ctypes.c_int64

    @contextlib.contextmanager
    def _hook(output_dir, device_ids):
        import jax

        jax.devices()
        if device_ids:
            ids = (ctypes.c_int64 * len(device_ids))(*device_ids)
            rc = lib.axon_start_nrt_profile(ids, len(device_ids))
        else:
            rc = lib.axon_start_nrt_profile(None, 0)
        if rc != 0:
            raise RuntimeError(f"axon_start_nrt_profile rc={rc}")
        try:
            yield
        finally:
            n = lib.axon_stop_nrt_profile(str(output_dir).encode())
            print(f"ntff shim: {n} file(s) written to {output_dir}")

    mod = types.ModuleType("antenv.axon_hooks")
    mod.get_axon_ntff_profile_hook = lambda: _hook
    mod.set_axon_ntff_profile_hook = lambda h: None
    import antenv

    sys.modules["antenv.axon_hooks"] = mod
    antenv.axon_hooks = mod


_install_ntff_shim()

def _split_waits(nc, max_waits=1):
    """Split >max_waits sync waits on one instruction into preceding
    wait-only drains (walrus setupSyncWait limit workaround)."""
    ctr = [0]
    for f in nc.m.functions:
        for blk in f.blocks:
            new_list = []
            for inst in blk.instructions:
                si = getattr(inst, "sync_info", None)
                if si is not None and si.on_wait and len(si.on_wait) > max_waits:
                    waits = list(si.on_wait)
                    extra, keep = waits[:-max_waits], waits[-max_waits:]
                    for wcond in extra:
                        ctr[0] += 1
                        nop = mybir.InstDrain(
                            name=f"waitsplit_{ctr[0]}", ins=[], outs=[]
                        )
                        nop.engine = inst.engine
                        nop.sync_info = mybir.SyncInfo(on_wait=[wcond], on_update=[])
                        new_list.append(nop)
                        nc.register_instruction(nop, overwrite=True)
                    si.on_wait = keep
                new_list.append(inst)
            blk.instructions = new_list
    return nc


B, D, H, W = 2, 64, 384, 768
N_CORES = 8
ROWS = H // N_CORES  # 48

A_ROW = 128 * 7 * 66
A_B = 60 * A_ROW
BND_Y = 128 * 13440
BND_B = 56 * BND_Y
O_ROW = 124 * 7 * 66
O_B = 48 * O_ROW

NBLK = 7          # x blocks, X0 = 124*c
NSLOT = 7         # ring depth (rows)
BSLOT = 9         # band ring depth (prefetch by 1)
TICKS = 59        # per batch-b sweep
D2, D3 = 6, 11    # stage2/stage3 tick lags (1 full tick of slack each)

LAST_EXEC_NS = [None]
LAST_RES = [None]


def _build():
    nc = bass.Bass()
    # a[b, r, p, c, d']: pre-blocked input rows, value = slab[x=124c+p, d']
    a = nc.dram_tensor("a", [2, 60, 128, 7, 66], F16, kind="ExternalInput")
    bands = nc.dram_tensor("bands", [2, 56, 128, 13440], F8, kind="ExternalInput")
    # o[b, y, m, c, d']: x = 124*c + m, d = d'-1 (host discards pads)
    o = nc.dram_tensor("o", [2, 48, 124, 7, 66], F16, kind="ExternalOutput")

    with TileContext(nc) as tc:
        with (
            tc.tile_pool(name="bandp", bufs=1) as bandp,
            tc.tile_pool(name="ringp", bufs=1) as ringp,
            tc.tile_pool(name="stgp", bufs=8) as stgp,
            tc.tile_pool(name="psp", bufs=8, space="PSUM") as psp,
        ):
            band_slots = [
                bandp.tile([128, 13440], F8, name=f"bnd{sl}", tag=f"bnd{sl}")
                for sl in range(BSLOT)
            ]
            a_ring = [
                ringp.tile([128, 7, 66], F16, name=f"a{sl}", tag=f"a{sl}")
                for sl in range(NSLOT)
            ]
            s1_ring = [
                ringp.tile([128, 7, 66], F16, name=f"s1_{sl}", tag=f"s1_{sl}")
                for sl in range(NSLOT)
            ]
            s2_ring = [
                ringp.tile([128, 7, 66], F16, name=f"s2_{sl}", tag=f"s2_{sl}")
                for sl in range(NSLOT)
            ]
            # zero pads once; assemblies rewrite the same interior region
            # every revisit so pads stay zero forever.
            for ring in (s1_ring, s2_ring):
                for sl in range(NSLOT):
                    nc.vector.memset(ring[sl][:], 0.0)
            # staging tiles carry the d-pad columns (zeroed once) so every
            # assembly DMA moves one contiguous 924B run per partition.
            stg_init = []
            for q in range(8):
                t = stgp.tile([128, 7, 66], F16, name="stgz", tag="stg")
                nc.vector.memset(t[:], 0.0)
                stg_init.append(t)

            def load_a(b, r):
                nc.scalar.dma_start(
                    out=a_ring[r % NSLOT][:],
                    in_=bass.AP(
                        tensor=a,
                        offset=b * A_B + r * A_ROW,
                        ap=[[462, 128], [1, 462]],
                    ),
                )

            def load_band(b, ybidx):
                nc.scalar.dma_start(
                    out=band_slots[ybidx % BSLOT][:],
                    in_=bass.AP(
                        tensor=bands,
                        offset=b * BND_B + ybidx * BND_Y,
                        ap=[[13440, 128], [1, 13440]],
                    ),
                )

            def stage(st, b, y):
                src = (a_ring, s1_ring, s2_ring)[st - 1]
                ybidx = y + (0, 2, 4)[st - 1]
                bslot = band_slots[ybidx % BSLOT]
                stg = stgp.tile([128, 7, 66], F16, name="stg", tag="stg")
                for c in range(NBLK):
                    ps = psp.tile([128, 64], F32, name="ps", tag="ps")
                    t = 0
                    for i in range(5):
                        srct = src[(y + i) % NSLOT]
                        for g in range(3):
                            off = (c * 15 + i * 3 + g) * 128
                            nc.tensor.matmul(
                                ps[:],
                                bslot[:, off : off + 128],
                                srct[:, c, g : g + 64],
                                start=(t == 0),
                                stop=(t == 14),
                            )
                            t += 1
                    nc.vector.tensor_scalar_mul(
                        out=stg[:, c, 1:65], in0=ps[:], scalar1=1.0 / WSCALE
                    )
                if st < 3:
                    dst = (None, s1_ring, s2_ring)[st][y % NSLOT]
                    # stage2's assembly rides the ACT queue to keep SP light
                    eng = nc.scalar if st == 2 else nc.sync
                    eng.dma_start(out=dst[2:126, :, :], in_=stg[0:124, :, :])
                    # halo pairs across block boundaries
                    eng.dma_start(out=dst[0:2, 1:7, :], in_=stg[122:124, 0:6, :])
                    eng.dma_start(out=dst[126:128, 0:6, :], in_=stg[0:2, 1:7, :])
                else:
                    nc.sync.dma_start(
                        out=bass.AP(
                            tensor=o,
                            offset=b * O_B + y * O_ROW,
                            ap=[[7 * 66, 124], [66, 7], [1, 66]],
                        ),
                        in_=stg[0:124, :, :],
                    )

            for b in range(2):
                for T in range(TICKS):
                    if T == 0:
                        load_band(b, 0)
                        load_band(b, 1)
                    elif T + 1 < 56:
                        load_band(b, T + 1)
                    if T == 0:
                        for r in range(6):
                            load_a(b, r)
                    elif T <= 54:
                        load_a(b, T + 5)
                    if T >= D2:
                        stage(2, b, T - D2)
                    if T >= D3:
                        stage(3, b, T - D3)
                    if T < 56:
                        stage(1, b, T)
    _split_waits(nc)
    return nc


_NC_CACHE = [None]


def _host_prep(input1, input2, k):
    s = k * ROWS
    slab = np.zeros((2, 60, 872, 66), np.float16)
    lo, hi = max(0, s - 6), min(H, s + 54)
    slab[:, lo - (s - 6) : hi - (s - 6), 2:770, 1:65] = (
        input1[:, :, lo:hi, :].transpose(0, 2, 3, 1).astype(np.float16)
    )
    # pre-blocked: a_pb[b, r, p, c, :] = slab[b, r, 124c + p, :]
    pidx = np.arange(128)[:, None] + 124 * np.arange(NBLK)[None, :]  # [128, 7]
    a_pb = slab[:, :, pidx, :]  # [2, 60, 128, 7, 66]

    w6 = input2.reshape(2, 3, 5, 5, H, W)
    wpad = np.zeros((2, 3, 5, 5, 56, 872), np.float32)
    ylo, yhi = max(0, s - 4), min(H, s + 52)
    wpad[:, :, :, :, ylo - (s - 4) : yhi - (s - 4), :768] = w6[:, :, :, :, ylo:yhi, :]

    import ml_dtypes

    Z = np.zeros((2, 56, 128, NBLK, 15, 128), ml_dtypes.float8_e4m3)
    xidx = 124 * np.arange(NBLK)[:, None] + np.arange(124)[None, :]  # [7,124]
    m = np.arange(124)
    for i in range(5):
        for g in range(3):
            ig = i * 3 + g
            for j in range(5):
                V = wpad[:, g, i, j][:, :, xidx] * WSCALE  # [2, 56, 7, 124]
                Z[:, :, m + j, :, ig, m] = V.transpose(3, 0, 1, 2).astype(
                    ml_dtypes.float8_e4m3
                )
    return {"a": a_pb, "bands": Z.reshape(2, 56, 128, 13440)}


def kernel(input1: np.ndarray, input2: np.ndarray) -> np.ndarray:
    input1 = np.asarray(input1, dtype=np.float32)
    input2 = np.asarray(input2, dtype=np.float32)
    if _NC_CACHE[0] is None:
        _NC_CACHE[0] = _build()
    nc = _NC_CACHE[0]

    in_maps = [_host_prep(input1, input2, k) for k in range(N_CORES)]

    trace = bool(os.environ.get("LGA3_TRACE"))
    try:
        res = bass_utils.run_bass_kernel_spmd(
            nc, in_maps, core_ids=list(range(N_CORES)), trace=trace
        )
    except ModuleNotFoundError:
        res = bass_utils.run_bass_kernel_spmd(
            nc, in_maps, core_ids=list(range(N_CORES)), trace=False
        )
    LAST_EXEC_NS[0] = res.exec_time_ns
    LAST_RES[0] = res

    out = np.empty((B, D, H, W), np.float32)
    for k in range(N_CORES):
        s = k * ROWS
        ok = res.results[k]["o"].astype(np.float32)  # [2, 48, 124, 7, 66]
        for c in range(NBLK):
            keep = 24 if c == 6 else 124
            out[:, :, s : s + ROWS, 124 * c : 124 * c + keep] = ok[
                :, :, 0:keep, c, 1:65
            ].transpose(0, 3, 1, 2)
    return out
